# revision 22
# baseline (speedup 1.0000x reference)
"""DeepSeek-V3 MoE routing kernel for Trainium2 (Bass/Tile), 8-core SPMD.

Reference semantics (per token, E=256 experts, G=8 groups of 32):
  scores = sigmoid(logits); swb = scores + bias
  group_score[g] = sum of top-2 of swb within group g
  keep top-4 groups; among kept experts take top-8 by swb
  s = scores * onehot(top8); out_vals = sort_desc(s)/(sum(s)+1e-20)*2.5
  out_idx = indices in descending-s order

Sharding: tokens split evenly across 8 NeuronCores (data parallel),
bias replicated.  Inside a core: tiles of 128 tokens (partition dim) x
256 experts (free dim), processed in groups of TB tiles so elementwise
work batches into wide DVE/Pool/ACT instructions.

Engine split: ACT = sigmoid + sign-select, Pool(GpSimd) = broadcast
adds/multiplies, DVE = reductions, match_replace, max8, find_index8.
Big tiles are reused in place (swb -> swbm -> sgn share one tile;
scores -> s share one tile) so three tile groups can be in flight.
"""

import numpy as np

T_FULL = 131072
E = 256
G = 8
EG = 32
N_CORES = 8
T_CORE = T_FULL // N_CORES
P = 128
NEG = -1.0e30
TB = 8  # tiles per batch group
SPLIT = 2  # tiles per Pool sub-op for swbm/s (pipelining granularity)


def build_bass(n_tokens: int):
    """Build the single-core Bass module processing [n_tokens, 256]."""
    from contextlib import ExitStack

    import concourse.bacc as bacc
    import concourse.mybir as mybir
    import concourse.tile as tile

    f32 = mybir.dt.float32
    A = mybir.AluOpType
    AX = mybir.AxisListType
    AF = mybir.ActivationFunctionType

    assert n_tokens % (P * TB) == 0
    n_groups = n_tokens // (P * TB)
    W = TB * E  # batched free width

    SW = SPLIT * E
    nc = bacc.Bacc("TRN2", target_bir_lowering=False, debug=False)

    logits_d = nc.dram_tensor("logits", [n_tokens, E], f32, kind="ExternalInput").ap()
    biasb_d = nc.dram_tensor("biasb", [P, W], f32, kind="ExternalInput").ap()
    idx_d = nc.dram_tensor("idx", [n_tokens, 8], mybir.dt.int32, kind="ExternalOutput").ap()
    vals_d = nc.dram_tensor("vals", [n_tokens, 8], f32, kind="ExternalOutput").ap()

    with tile.TileContext(nc) as tc, ExitStack() as ctx:
        setup = ctx.enter_context(tc.tile_pool(name="setup", bufs=1))
        big = ctx.enter_context(tc.tile_pool(name="big", bufs=4))
        small = ctx.enter_context(tc.tile_pool(name="small", bufs=3))

        # bias pre-broadcast on host: [128, TB*256]
        bias_bc = setup.tile([P, W], f32)
        nc.sync.dma_start(bias_bc[:], biasb_d)
        negc = setup.tile([P, 1], f32)
        nc.vector.memset(negc[:], NEG)

        def phase_a1(i):
            """Group front: load .. group-mask launch (ends issuing the
            Pool swbm adds, whose latency the caller hides by emitting
            the previous group's phase_b next)."""
            rows = slice(i * P * TB, (i + 1) * P * TB)
            # DRAM view: [p, j, e] with token = i*P*TB + j*P + p
            dview = logits_d[rows, :].rearrange("(j p) e -> p j e", p=P)

            # scores tile: logits in, sigmoid in place, later s = scores*sgn
            scores = big.tile([P, W], f32, tag="scores")
            nc.sync.dma_start(scores[:].rearrange("p (j e) -> p j e", j=TB), dview)
            nc.scalar.activation(scores[:], scores[:], AF.Sigmoid)

            # swb tile: swb = scores + bias; later swbm, then sgn in place
            swb = big.tile([P, W], f32, tag="swb")
            nc.gpsimd.tensor_add(swb[:], scores[:], bias_bc[:])
            swb4 = swb[:].rearrange("p (j g e) -> p j g e", j=TB, g=G)

            m1 = small.tile([P, TB * G], f32, tag="m1")
            nc.vector.tensor_reduce(m1[:].rearrange("p (j g) -> p j g", j=TB),
                                    swb4, axis=AX.X, op=A.max)

            swb2 = big.tile([P, W], f32, tag="swb2")
            for j in range(TB):
                nc.vector.match_replace(
                    out=swb2[:, j * E:(j + 1) * E],
                    in_to_replace=m1[:, j * G:(j + 1) * G],
                    in_values=swb[:, j * E:(j + 1) * E],
                    imm_value=NEG)

            m2 = small.tile([P, TB * G], f32, tag="m2")
            nc.vector.tensor_reduce(
                m2[:].rearrange("p (j g) -> p j g", j=TB),
                swb2[:].rearrange("p (j g e) -> p j g e", j=TB, g=G),
                axis=AX.X, op=A.max)

            gs = small.tile([P, TB * G], f32, tag="gs")
            nc.gpsimd.tensor_add(gs[:], m1[:], m2[:])

            gm8 = small.tile([P, TB * 8], f32, tag="gm8")
            for j in range(TB):
                nc.vector.max(out=gm8[:, j * 8:(j + 1) * 8],
                              in_=gs[:, j * G:(j + 1) * G])

            # cmp = 1.0 where group NOT selected (gs < 4th-largest)
            tg = gm8[:, 3::8]  # [P, TB]
            cmp = small.tile([P, TB * G], f32, tag="cmp")
            nc.vector.tensor_tensor(
                out=cmp[:].rearrange("p (j g) -> p j g", j=TB),
                in0=gs[:].rearrange("p (j g) -> p j g", j=TB),
                in1=tg.to_broadcast([P, TB, G]),
                op=A.is_lt)
            goff = small.tile([P, TB * G], f32, tag="goff")
            nc.gpsimd.tensor_tensor(goff[:], cmp[:],
                                    negc[:].to_broadcast([P, TB * G]), op=A.mult)

            # swbm = swb + goff (in place over swb; masked groups -> -1e30)
            for j in range(0, TB, SPLIT):
                sl = slice(j * E, j * E + SW)
                nc.gpsimd.tensor_add(
                    swb[:, sl].rearrange("p (j g e) -> p j g e", j=SPLIT, g=G),
                    swb[:, sl].rearrange("p (j g e) -> p j g e", j=SPLIT, g=G),
                    goff[:, j * G:(j + SPLIT) * G]
                    .rearrange("p (j g) -> p j g", j=SPLIT)
                    .to_broadcast([P, SPLIT, G, EG]))
            return scores, swb

        def phase_a2(i, scores, swb):
            """Group mid: biased top-8, sign-select, s = scores*sgn."""
            v8b = small.tile([P, TB * 8], f32, tag="v8b")
            for j in range(TB):
                nc.vector.max(out=v8b[:, j * 8:(j + 1) * 8],
                              in_=swb[:, j * E:(j + 1) * E])

            # negated next-below-t8 threshold: -(t8 - |t8|*1.5*2^-23) =
            # max(t8*(c-1), t8*(-c-1)) with c = 1.5*2^-23, computed without
            # abs: nt8p = max(t8*(c-1), -t8*(c+1)).
            c = 1.5 * 2.0 ** -23
            t8lo = small.tile([P, TB], f32, tag="t8lo")
            nc.scalar.mul(t8lo[:], v8b[:, 7::8], c - 1.0)
            t8hi = small.tile([P, TB], f32, tag="t8hi")
            nc.scalar.mul(t8hi[:], v8b[:, 7::8], -c - 1.0)
            nt8p = small.tile([P, TB], f32, tag="nt8p")
            nc.vector.tensor_tensor(nt8p[:], t8lo[:], t8hi[:], op=A.max)

            # sgn = Sign(swbm + nt8p) in {-1,0,+1}, +1 exactly at selected
            # top-8 positions (in place over swbm).
            for j in range(TB):
                nc.scalar.activation(
                    swb[:, j * E:(j + 1) * E], swb[:, j * E:(j + 1) * E],
                    AF.Sign, bias=nt8p[:, j:j + 1], scale=1.0)

            # s = scores * sgn (in place over scores): selected scores stay
            # positive, all else <= 0 and never enters the final top-8.
            for j in range(0, TB, SPLIT):
                sl = slice(j * E, j * E + SW)
                nc.gpsimd.tensor_tensor(scores[:, sl], scores[:, sl],
                                        swb[:, sl], op=A.mult)
            return scores

        def phase_b(i, scores):
            """Group back half: final top-8 over s, indices, normalize."""
            rows = slice(i * P * TB, (i + 1) * P * TB)
            v8u = small.tile([P, TB * 8], f32, tag="v8u")
            for j in range(TB):
                nc.vector.max(out=v8u[:, j * 8:(j + 1) * 8],
                              in_=scores[:, j * E:(j + 1) * E])

            idx8 = small.tile([P, TB * 8], mybir.dt.uint32, tag="idx8")
            for j in range(TB):
                nc.vector.max_index(out=idx8[:, j * 8:(j + 1) * 8],
                                    in_max=v8u[:, j * 8:(j + 1) * 8],
                                    in_values=scores[:, j * E:(j + 1) * E])

            # vals = v8u * (2.5 / ssum):  rec25 = 1/(ssum*0.4)
            ssum = small.tile([P, TB], f32, tag="ssum")
            nc.vector.tensor_reduce(ssum[:],
                                    v8u[:].rearrange("p (j k) -> p j k", j=TB),
                                    axis=AX.X, op=A.add)
            ssum4 = small.tile([P, TB], f32, tag="ssum4")
            nc.scalar.mul(ssum4[:], ssum[:], 0.4)
            rec = small.tile([P, TB], f32, tag="rec")
            nc.vector.reciprocal(rec[:], ssum4[:])

            vals8 = small.tile([P, TB * 8], f32, tag="vals8")
            nc.gpsimd.tensor_tensor(
                vals8[:].rearrange("p (j k) -> p j k", j=TB),
                v8u[:].rearrange("p (j k) -> p j k", j=TB),
                rec[:].to_broadcast([P, TB, 8]), op=A.mult)

            oi = idx_d[rows, :].rearrange("(j p) k -> p j k", p=P)
            ov = vals_d[rows, :].rearrange("(j p) k -> p j k", p=P)
            nc.sync.dma_start(
                oi, idx8[:].bitcast(mybir.dt.int32).rearrange("p (j k) -> p j k", j=TB))
            nc.sync.dma_start(ov, vals8[:].rearrange("p (j k) -> p j k", j=TB))

        # one-group software pipeline skew: the previous group's phase_b
        # (DVE-heavy) is emitted between this group's mask launch (a1,
        # ends with Pool swbm adds) and biased-top8 (a2), hiding both the
        # Pool swbm latency and the sign/s-mult latency of a2.
        prev = None
        for i in range(n_groups):
            sc, sw = phase_a1(i)
            if prev is not None:
                phase_b(i - 1, prev)
            prev = phase_a2(i, sc, sw)
        phase_b(n_groups - 1, prev)

    nc.compile()
    return nc


_NC_CACHE = {}


def _get_nc(n_tokens: int):
    if n_tokens not in _NC_CACHE:
        _NC_CACHE[n_tokens] = build_bass(n_tokens)
    return _NC_CACHE[n_tokens]


def run_spmd(nc, logits, bias, trace=False):
    from concourse import bass_utils

    n = logits.shape[0] // N_CORES
    biasb = np.ascontiguousarray(
        np.broadcast_to(np.tile(bias, TB)[None, :], (P, TB * E)).astype(np.float32))
    in_maps = [
        {"logits": np.ascontiguousarray(logits[c * n:(c + 1) * n]),
         "biasb": biasb}
        for c in range(N_CORES)
    ]
    res = bass_utils.run_bass_kernel_spmd(nc, in_maps, list(range(N_CORES)),
                                          trace=trace)
    idx = np.concatenate([r["idx"] for r in res.results], axis=0)
    vals = np.concatenate([r["vals"] for r in res.results], axis=0)
    return (idx.astype(np.int32), vals.astype(np.float32)), res


def kernel(logits, e_score_correction_bias):
    logits = np.asarray(logits, dtype=np.float32)
    bias = np.asarray(e_score_correction_bias, dtype=np.float32)
    assert logits.shape == (T_FULL, E)
    nc = _get_nc(T_CORE)
    (idx, vals), _ = run_spmd(nc, logits, bias)
    return idx, vals


# revision 23
# speedup vs baseline: 1.2124x; 1.2124x over previous
"""DeepSeek-V3 MoE routing kernel for Trainium2 (Bass/Tile), 8-core SPMD.

Reference semantics (per token, E=256 experts, G=8 groups of 32):
  scores = sigmoid(logits); swb = scores + bias
  group_score[g] = sum of top-2 of swb within group g
  keep top-4 groups; among kept experts take top-8 by swb
  s = scores * onehot(top8); out_vals = sort_desc(s)/(sum(s)+1e-20)*2.5
  out_idx = indices in descending-s order

Sharding: tokens split evenly across 8 NeuronCores (data parallel),
bias replicated.  Inside a core: tiles of 128 tokens (partition dim) x
256 experts (free dim), processed in groups of TB tiles so elementwise
work batches into wide DVE/Pool/ACT instructions.

Engine split: ACT = sigmoid + sign-select, Pool(GpSimd) = broadcast
adds/multiplies, DVE = reductions, match_replace, max8, find_index8.
Big tiles are reused in place (swb -> swbm -> sgn share one tile;
scores -> s share one tile) so three tile groups can be in flight.
"""

import numpy as np

T_FULL = 131072
E = 256
G = 8
EG = 32
N_CORES = 8
T_CORE = T_FULL // N_CORES
P = 128
NEG = -1.0e30
TB = 8  # tiles per batch group
SPLIT = 2  # tiles per Pool sub-op for swbm/s (pipelining granularity)


def build_bass(n_tokens: int):
    """Build the single-core Bass module processing [n_tokens, 256]."""
    from contextlib import ExitStack

    import concourse.bacc as bacc
    import concourse.mybir as mybir
    import concourse.tile as tile

    f32 = mybir.dt.float32
    A = mybir.AluOpType
    AX = mybir.AxisListType
    AF = mybir.ActivationFunctionType

    assert n_tokens % (P * TB) == 0
    n_groups = n_tokens // (P * TB)
    W = TB * E  # batched free width

    SW = SPLIT * E
    nc = bacc.Bacc("TRN2", target_bir_lowering=False, debug=False)

    logits_d = nc.dram_tensor("logits", [n_tokens, E], f32, kind="ExternalInput").ap()
    biasb_d = nc.dram_tensor("biasb", [P, W], f32, kind="ExternalInput").ap()
    idx_d = nc.dram_tensor("idx", [n_tokens, 8], mybir.dt.int32, kind="ExternalOutput").ap()
    vals_d = nc.dram_tensor("vals", [n_tokens, 8], f32, kind="ExternalOutput").ap()

    with tile.TileContext(nc) as tc, ExitStack() as ctx:
        setup = ctx.enter_context(tc.tile_pool(name="setup", bufs=1))
        big = ctx.enter_context(tc.tile_pool(name="big", bufs=4))
        small = ctx.enter_context(tc.tile_pool(name="small", bufs=3))

        # bias pre-broadcast on host: [128, TB*256]
        bias_bc = setup.tile([P, W], f32)
        nc.sync.dma_start(bias_bc[:], biasb_d)
        negc = setup.tile([P, 1], f32)
        nc.vector.memset(negc[:], NEG)

        def phase_a1(i):
            """Group front: load .. group-mask launch (ends issuing the
            Pool swbm adds, whose latency the caller hides by emitting
            the previous group's phase_b next)."""
            rows = slice(i * P * TB, (i + 1) * P * TB)
            # DRAM view: [p, j, e] with token = i*P*TB + j*P + p
            dview = logits_d[rows, :].rearrange("(j p) e -> p j e", p=P)

            # scores tile: logits in, sigmoid in place, later s = scores*sgn
            scores = big.tile([P, W], f32, tag="scores")
            nc.sync.dma_start(scores[:].rearrange("p (j e) -> p j e", j=TB), dview)
            nc.scalar.activation(scores[:], scores[:], AF.Sigmoid)

            # swb tile: swb = scores + bias; later swbm, then sgn in place
            swb = big.tile([P, W], f32, tag="swb")
            nc.gpsimd.tensor_add(swb[:], scores[:], bias_bc[:])
            swb4 = swb[:].rearrange("p (j g e) -> p j g e", j=TB, g=G)

            m1 = small.tile([P, TB * G], f32, tag="m1")
            nc.vector.tensor_reduce(m1[:].rearrange("p (j g) -> p j g", j=TB),
                                    swb4, axis=AX.X, op=A.max)

            swb2 = big.tile([P, W], f32, tag="swb2")
            for j in range(TB):
                nc.vector.match_replace(
                    out=swb2[:, j * E:(j + 1) * E],
                    in_to_replace=m1[:, j * G:(j + 1) * G],
                    in_values=swb[:, j * E:(j + 1) * E],
                    imm_value=NEG)

            m2 = small.tile([P, TB * G], f32, tag="m2")
            nc.vector.tensor_reduce(
                m2[:].rearrange("p (j g) -> p j g", j=TB),
                swb2[:].rearrange("p (j g e) -> p j g e", j=TB, g=G),
                axis=AX.X, op=A.max)

            gs = small.tile([P, TB * G], f32, tag="gs")
            nc.vector.tensor_add(gs[:], m1[:], m2[:])

            gm8 = small.tile([P, TB * 8], f32, tag="gm8")
            for j in range(TB):
                nc.vector.max(out=gm8[:, j * 8:(j + 1) * 8],
                              in_=gs[:, j * G:(j + 1) * G])

            # cmp = 1.0 where group NOT selected (gs < 4th-largest)
            tg = gm8[:, 3::8]  # [P, TB]
            cmp = small.tile([P, TB * G], f32, tag="cmp")
            nc.vector.tensor_tensor(
                out=cmp[:].rearrange("p (j g) -> p j g", j=TB),
                in0=gs[:].rearrange("p (j g) -> p j g", j=TB),
                in1=tg.to_broadcast([P, TB, G]),
                op=A.is_lt)
            goff = small.tile([P, TB * G], f32, tag="goff")
            nc.gpsimd.tensor_tensor(goff[:], cmp[:],
                                    negc[:].to_broadcast([P, TB * G]), op=A.mult)

            # swbm = swb + goff (in place over swb; masked groups -> -1e30)
            for j in range(0, TB, SPLIT):
                sl = slice(j * E, j * E + SW)
                nc.gpsimd.tensor_add(
                    swb[:, sl].rearrange("p (j g e) -> p j g e", j=SPLIT, g=G),
                    swb[:, sl].rearrange("p (j g e) -> p j g e", j=SPLIT, g=G),
                    goff[:, j * G:(j + SPLIT) * G]
                    .rearrange("p (j g) -> p j g", j=SPLIT)
                    .to_broadcast([P, SPLIT, G, EG]))
            return scores, swb

        def phase_a2(i, scores, swb):
            """Group mid: biased top-8, sign-select, s = scores*sgn."""
            v8b = small.tile([P, TB * 8], f32, tag="v8b")
            for j in range(TB):
                nc.vector.max(out=v8b[:, j * 8:(j + 1) * 8],
                              in_=swb[:, j * E:(j + 1) * E])

            # negated next-below-t8 threshold: -(t8 - |t8|*1.5*2^-23) =
            # max(t8*(c-1), t8*(-c-1)) with c = 1.5*2^-23, computed without
            # abs: nt8p = max(t8*(c-1), -t8*(c+1)).
            c = 1.5 * 2.0 ** -23
            t8lo = small.tile([P, TB], f32, tag="t8lo")
            nc.scalar.mul(t8lo[:], v8b[:, 7::8], c - 1.0)
            t8hi = small.tile([P, TB], f32, tag="t8hi")
            nc.scalar.mul(t8hi[:], v8b[:, 7::8], -c - 1.0)
            nt8p = small.tile([P, TB], f32, tag="nt8p")
            nc.vector.tensor_tensor(nt8p[:], t8lo[:], t8hi[:], op=A.max)

            # sgn = Sign(swbm + nt8p) in {-1,0,+1}, +1 exactly at selected
            # top-8 positions (in place over swbm).
            for j in range(TB):
                nc.scalar.activation(
                    swb[:, j * E:(j + 1) * E], swb[:, j * E:(j + 1) * E],
                    AF.Sign, bias=nt8p[:, j:j + 1], scale=1.0)

            # s = scores * sgn (in place over scores): selected scores stay
            # positive, all else <= 0 and never enters the final top-8.
            for j in range(0, TB, SPLIT):
                sl = slice(j * E, j * E + SW)
                nc.gpsimd.tensor_tensor(scores[:, sl], scores[:, sl],
                                        swb[:, sl], op=A.mult)
            return scores

        def phase_b(i, scores):
            """Group back half: final top-8 over s, indices, normalize."""
            rows = slice(i * P * TB, (i + 1) * P * TB)
            v8u = small.tile([P, TB * 8], f32, tag="v8u")
            for j in range(TB):
                nc.vector.max(out=v8u[:, j * 8:(j + 1) * 8],
                              in_=scores[:, j * E:(j + 1) * E])

            idx8 = small.tile([P, TB * 8], mybir.dt.uint32, tag="idx8")
            for j in range(TB):
                nc.vector.max_index(out=idx8[:, j * 8:(j + 1) * 8],
                                    in_max=v8u[:, j * 8:(j + 1) * 8],
                                    in_values=scores[:, j * E:(j + 1) * E])

            # vals = v8u * (2.5 / ssum):  rec25 = 1/(ssum*0.4)
            ssum = small.tile([P, TB], f32, tag="ssum")
            nc.vector.tensor_reduce(ssum[:],
                                    v8u[:].rearrange("p (j k) -> p j k", j=TB),
                                    axis=AX.X, op=A.add)
            ssum4 = small.tile([P, TB], f32, tag="ssum4")
            nc.scalar.mul(ssum4[:], ssum[:], 0.4)
            rec = small.tile([P, TB], f32, tag="rec")
            nc.vector.reciprocal(rec[:], ssum4[:])

            vals8 = small.tile([P, TB * 8], f32, tag="vals8")
            nc.gpsimd.tensor_tensor(
                vals8[:].rearrange("p (j k) -> p j k", j=TB),
                v8u[:].rearrange("p (j k) -> p j k", j=TB),
                rec[:].to_broadcast([P, TB, 8]), op=A.mult)

            oi = idx_d[rows, :].rearrange("(j p) k -> p j k", p=P)
            ov = vals_d[rows, :].rearrange("(j p) k -> p j k", p=P)
            nc.sync.dma_start(
                oi, idx8[:].bitcast(mybir.dt.int32).rearrange("p (j k) -> p j k", j=TB))
            nc.sync.dma_start(ov, vals8[:].rearrange("p (j k) -> p j k", j=TB))

        # one-group software pipeline skew: the previous group's phase_b
        # (DVE-heavy) is emitted between this group's mask launch (a1,
        # ends with Pool swbm adds) and biased-top8 (a2), hiding both the
        # Pool swbm latency and the sign/s-mult latency of a2.
        prev = None
        for i in range(n_groups):
            sc, sw = phase_a1(i)
            if prev is not None:
                phase_b(i - 1, prev)
            prev = phase_a2(i, sc, sw)
        phase_b(n_groups - 1, prev)

    nc.compile()
    return nc


_NC_CACHE = {}


def _get_nc(n_tokens: int):
    if n_tokens not in _NC_CACHE:
        _NC_CACHE[n_tokens] = build_bass(n_tokens)
    return _NC_CACHE[n_tokens]


def run_spmd(nc, logits, bias, trace=False):
    from concourse import bass_utils

    n = logits.shape[0] // N_CORES
    biasb = np.ascontiguousarray(
        np.broadcast_to(np.tile(bias, TB)[None, :], (P, TB * E)).astype(np.float32))
    in_maps = [
        {"logits": np.ascontiguousarray(logits[c * n:(c + 1) * n]),
         "biasb": biasb}
        for c in range(N_CORES)
    ]
    res = bass_utils.run_bass_kernel_spmd(nc, in_maps, list(range(N_CORES)),
                                          trace=trace)
    idx = np.concatenate([r["idx"] for r in res.results], axis=0)
    vals = np.concatenate([r["vals"] for r in res.results], axis=0)
    return (idx.astype(np.int32), vals.astype(np.float32)), res


def kernel(logits, e_score_correction_bias):
    logits = np.asarray(logits, dtype=np.float32)
    bias = np.asarray(e_score_correction_bias, dtype=np.float32)
    assert logits.shape == (T_FULL, E)
    nc = _get_nc(T_CORE)
    (idx, vals), _ = run_spmd(nc, logits, bias)
    return idx, vals
